# revision 44
# baseline (speedup 1.0000x reference)
"""Multi-Head Latent Attention (MLA) Trainium2 kernel, 8-core head-sharded.

Layout: all matmuls run with the contraction dim on partitions
("transposed world"); x and every weight are pre-transposed on the host.
Heads are sharded 2-per-core; each core emits a bf16 partial out.T (its
heads' contribution to the output projection), summed and transposed on
the host (rel err ~4e-3, harness gate 2e-2).

Precision: bf16 operands everywhere on the PE (same 1 cycle/column rate
as fp32r but fast FWL weight loads and half the DMA/SBUF); all PSUM
accumulation is fp32. W_DQ is SBUF-resident in bf16 (no weight stream).
k_R/q_R are zero-padded to 128 partitions (64-row fp32r moving operands
ran at half rate; also keeps bf16 FWL eligible), and W_KR columns are
host-duplicated so the kr matmul has a full 128-wide stationary. The
softmax denominator is a matmul against an all-ones [128,128] stationary,
which lands the row-sum broadcast across all 128 PSUM partitions - the
reciprocal (reciprocal_approx_fast) is then multiplied in directly with
no broadcast matmul.

Pipeline per query block qb: stage2(qb, q-rope first) -> attention(qb)
-> stage1(qb+1) -> stage5(qb), so the reciprocal chain and next-block
DMAs hide under stage-1 matmuls. DMA issue is spread across the sync /
gpsimd / scalar queues to avoid head-of-line blocking of the out-DMAs.
"""
import sys

sys.path.insert(0, "/opt/trn_rl_repo")

import numpy as np

import concourse.bass as bass
import concourse.tile as tile
from concourse import bacc, mybir
from concourse.bass_utils import run_bass_kernel_spmd

F32 = mybir.dt.float32
F32R = mybir.dt.float32r
BF16 = mybir.dt.bfloat16
AF = mybir.ActivationFunctionType
OP = mybir.AluOpType

N_CORES = 8
S = 2048          # sequence length
DM = 2048         # d_model
DL = 512          # d_latent
H = 16            # total heads
HC = H // N_CORES  # heads per core (2)
DH = 128          # head dim (content)
DHR = 64          # head dim (rope)
QB = 512          # query block
NQB = S // QB     # 4
KPB = QB // 128   # key chunks per query block (4)
NMC = DM // 128   # 16 model chunks
NLC = DL // 128   # 4 latent chunks
NKC = S // 128    # 16 key chunks
THETA = 10000.0

SCALE = float(1.0 / np.sqrt(np.float32(DH + DHR)))
E_HI = float(np.exp(np.float64(80.0) * SCALE))
E_LO = float(np.exp(np.float64(-80.0) * SCALE))

# Set by test.py to profile; harness path leaves these untouched.
TRACE = False
TRACE_KWARGS = {}
LAST_EXEC_TIME_NS = None
LAST_RESULTS = None

_CACHE = {}
MM_LABELS = {}


def _lbl(inst, label):
    try:
        MM_LABELS[inst.ins.name] = label
    except Exception:
        try:
            MM_LABELS[inst.name] = label
        except Exception:
            pass
    return inst


def _build():
    nc = bacc.Bacc("TRN2", target_bir_lowering=False, debug=False,
                   enable_asserts=True, num_devices=N_CORES)

    def din(name, shape, dt=F32R):
        return nc.dram_tensor(name, shape, dt, kind="ExternalInput").ap()

    d = {
        "xT": din("xT", [DM, S], BF16),
        "wdqT": din("wdqT", [DM, DL], BF16),
        "wdkvT": din("wdkvT", [DM, DL], BF16),
        "wkrT": din("wkrT", [DM, 128], BF16),
        "wuqT": din("wuqT", [DL, HC * DH], BF16),
        "wqrT": din("wqrT", [DL, HC * DHR], BF16),
        "wukT": din("wukT", [DL, HC * DH], BF16),
        "wuvT": din("wuvT", [DL, HC * DH], BF16),
        "woT": din("woT", [HC * DH, DM], BF16),
        "ones128": din("ones128", [128, 128], BF16),
        "ones1": din("ones1", [1, 128]),
        "masktri": din("masktri", [128, 128], F32),
        "zeros64": din("zeros64", [64, S], BF16),
        "cs1": din("cs1", [DHR, S], F32),
        "cs2": din("cs2", [DHR, S], F32),
        "outT": nc.dram_tensor("outT", [DM, S], BF16,
                               kind="ExternalOutput").ap(),
    }
    with tile.TileContext(nc) as tc:
        import contextlib
        with contextlib.ExitStack() as ctx:
            _kernel_body(ctx, tc, nc, d)
    nc.compile()
    return nc


def _kernel_body(ctx, tc, nc, d):
    wts = ctx.enter_context(tc.tile_pool(name="wts", bufs=1))
    kvp = ctx.enter_context(tc.tile_pool(name="kvp", bufs=1))
    xtp = ctx.enter_context(tc.tile_pool(name="xtp", bufs=1))
    lat = ctx.enter_context(tc.tile_pool(name="lat", bufs=1))
    prj = ctx.enter_context(tc.tile_pool(name="prj", bufs=1))
    smp = ctx.enter_context(tc.tile_pool(name="smp", bufs=1))
    o5p = ctx.enter_context(tc.tile_pool(name="o5p", bufs=8))
    # PSUM: stage-1 dedicated (3) + work rotation (2) + attn (2) + sums (1)
    ps_s1 = ctx.enter_context(tc.tile_pool(name="ps_s1", bufs=1, space="PSUM"))
    ps_at = ctx.enter_context(tc.tile_pool(name="ps_at", bufs=2, space="PSUM"))
    ps_sm = ctx.enter_context(tc.tile_pool(name="ps_sm", bufs=2, space="PSUM"))

    s1rot = [0]

    def s1tile(shape, name):
        t = ps_s1.tile(shape, F32, tag=f"s1{s1rot[0] % 4}", name=name)
        s1rot[0] += 1
        return t

    # ---- small persistent loads ----
    o128_t = wts.tile([128, 128], BF16, name="o128")
    o1_t = wts.tile([1, 128], F32R, name="o1")
    nc.scalar.dma_start(o128_t[:], d["ones128"][:, :])
    nc.scalar.dma_start(o1_t[:], d["ones1"][:, :])
    mask_t = wts.tile([128, 128], F32, name="masktri")
    nc.scalar.dma_start(mask_t[:], d["masktri"][:, :])

    wkr_t = [wts.tile([128, 128], BF16, name=f"wkr{m}") for m in range(NMC)]
    for m in range(NMC):
        nc.gpsimd.dma_start(wkr_t[m][:], d["wkrT"][m * 128:(m + 1) * 128, :])
    wdq_t = [wts.tile([128, DL], BF16, name=f"wdq{m}") for m in range(NMC)]
    for m in range(NMC):
        nc.gpsimd.dma_start(wdq_t[m][:], d["wdqT"][m * 128:(m + 1) * 128, :])
    wdkv_t = [wts.tile([128, DL], BF16, name=f"wdkv{m}") for m in range(NMC)]
    wuq_t = [wts.tile([128, HC * DH], BF16, name=f"wuq{l}") for l in range(NLC)]
    wqr_t = [wts.tile([128, HC * DHR], BF16, name=f"wqr{l}") for l in range(NLC)]
    wuk_t = [wts.tile([128, HC * DH], BF16, name=f"wuk{l}") for l in range(NLC)]
    wuv_t = [wts.tile([128, HC * DH], BF16, name=f"wuv{l}") for l in range(NLC)]
    wo_t = [wts.tile([128, DM], BF16, name=f"wo{h}") for h in range(HC)]

    def emit_wdkv_dmas():
        for m in range(NMC):
            nc.gpsimd.dma_start(wdkv_t[m][:], d["wdkvT"][m * 128:(m + 1) * 128, :])

    def emit_proj_dmas():
        for l in range(NLC):
            nc.gpsimd.dma_start(wuk_t[l][:], d["wukT"][l * 128:(l + 1) * 128, :])
            nc.gpsimd.dma_start(wuv_t[l][:], d["wuvT"][l * 128:(l + 1) * 128, :])
            nc.gpsimd.dma_start(wuq_t[l][:], d["wuqT"][l * 128:(l + 1) * 128, :])
            nc.gpsimd.dma_start(wqr_t[l][:], d["wqrT"][l * 128:(l + 1) * 128, :])

    def emit_wo_dmas():
        for h in range(HC):
            nc.gpsimd.dma_start(wo_t[h][:], d["woT"][h * 128:(h + 1) * 128, :])

    # ---- persistent per-sequence state ----
    kct = [kvp.tile([128, S], BF16, name=f"kct{h}") for h in range(HC)]
    # krt/qrt are zero-padded to 128 partitions: a 64-partition moving
    # operand runs fp32r matmuls at half rate.
    krt = kvp.tile([128, S], BF16, name="krt")
    nc.scalar.dma_start(krt[DHR:128, :], d["zeros64"][:, :])
    qrt = [kvp.tile([128, QB], BF16, name=f"qrt{h}") for h in range(HC)]
    for h in range(HC):
        nc.scalar.dma_start(qrt[h][DHR:128, :], d["zeros64"][:, 0:QB])
    vt = [kvp.tile([128, HC * DH], BF16, name=f"vt{k}") for k in range(NKC)]



    def rope(raw_pt, out_ap, cs1s, cs2s, tag):
        """raw_pt: PSUM tile holding [64, QB] pre-rope rows; out_ap:
        bf16 dest [64, QB]. Swap-halves DMA reads PSUM directly; the cs2
        leg runs on gpsimd so the two products overlap."""
        raw = smp.tile([DHR, QB], F32, tag="rope_srcc", name=f"rc_{tag}")
        nc.scalar.copy(raw[:], raw_pt[0:DHR, :])
        rsw = smp.tile([DHR, QB], F32, tag="rope_swp", name=f"rs_{tag}")
        nc.sync.dma_start(rsw[0:32, :], raw[32:64, :])
        nc.sync.dma_start(rsw[32:64, :], raw[0:32, :])
        rawm = smp.tile([DHR, QB], F32, tag="rope_raw", name=f"rr_{tag}")
        nc.vector.tensor_tensor(rawm[:], raw_pt[0:DHR, :], cs1s[:], op=OP.mult)
        nc.gpsimd.tensor_tensor(rsw[:], rsw[:], cs2s[:], op=OP.mult)
        nc.vector.tensor_tensor(out_ap, rawm[:], rsw[:], op=OP.add)

    def stage1(qb, mid_hook=None):
        """Latents in 4 mc-major passes; x and all stage-1 weights are
        bf16 (resident W_DQ, no weight streaming)."""
        qsl = slice(qb * QB, (qb + 1) * QB)
        xt = [xtp.tile([128, QB], BF16, tag=f"xt{m}", name=f"xt{m}_{qb}")
              for m in range(NMC)]
        for m in range(NMC):
            nc.sync.dma_start(xt[m][:], d["xT"][m * 128:(m + 1) * 128, qsl])
        cs1s = smp.tile([DHR, QB], F32, tag="cs1s", bufs=1, name=f"cs1s{qb}")
        cs2s = smp.tile([DHR, QB], F32, tag="cs2s", bufs=1, name=f"cs2s{qb}")
        nc.gpsimd.dma_start(cs1s[:], d["cs1"][:, qsl])
        nc.gpsimd.dma_start(cs2s[:], d["cs2"][:, qsl])

        ckv = [lat.tile([128, QB], BF16, tag=f"ckv{l}", name=f"ckv{l}_{qb}")
               for l in range(NLC)]
        cq = [lat.tile([128, QB], BF16, tag=f"cq{l}", name=f"cq{l}_{qb}")
              for l in range(NLC)]
        eng_tgl = [0]

        def copy_out(dst, src):
            (nc.vector.tensor_copy if eng_tgl[0] % 2 == 0
             else nc.scalar.copy)(dst, src)
            eng_tgl[0] += 1

        plan = [
            [("kr", None), ("cq", 0), ("cq", 1)],
            [("cq", 2), ("cq", 3)],
            [("ckv", 0), ("ckv", 1)],
            [("ckv", 2), ("ckv", 3)],
        ]
        for pi, groups in enumerate(plan):
            pts = []
            for gi, (kind, idx) in enumerate(groups):
                pts.append(s1tile([128, QB], f"p{pi}{gi}_{qb}"))
            for m in range(NMC):
                for gi, (kind, idx) in enumerate(groups):
                    if kind == "kr":
                        st_ap, label = wkr_t[m][:], "s1_kr"
                    elif kind == "cq":
                        st_ap = wdq_t[m][:, idx * 128:(idx + 1) * 128]
                        label = "s1_cq"
                    else:
                        st_ap = wdkv_t[m][:, idx * 128:(idx + 1) * 128]
                        label = "s1_ckv"
                    _lbl(nc.tensor.matmul(pts[gi][:], st_ap, xt[m][:],
                                          start=(m == 0),
                                          stop=(m == NMC - 1)), label)
            for gi, (kind, idx) in enumerate(groups):
                if kind == "kr":
                    rope(pts[gi], krt[0:DHR, qsl], cs1s, cs2s, f"kr{qb}")
                elif kind == "cq":
                    copy_out(cq[idx][:], pts[gi][:])
                else:
                    copy_out(ckv[idx][:], pts[gi][:])
            if pi == 1 and mid_hook is not None:
                mid_hook()
        return cq, ckv, cs1s, cs2s

    def stage2(qb, cq, ckv, cs1s, cs2s):
        qsl = slice(qb * QB, (qb + 1) * QB)
        # q_C / q_R per head
        qct = [prj.tile([128, QB], BF16, tag=f"qct{h}", bufs=2, name=f"qct{h}_{qb}")
               for h in range(HC)]
        for h in range(HC):
            pqr = s1tile([DHR, QB], f"pqr{h}_{qb}")
            for l in range(NLC):
                _lbl(nc.tensor.matmul(pqr[:], wqr_t[l][:, h * DHR:(h + 1) * DHR],
                                 cq[l][:], start=(l == 0), stop=(l == NLC - 1)), "s2_qr")
            rope(pqr, qrt[h][0:DHR, :], cs1s, cs2s, f"qr{h}_{qb}")
        for h in range(HC):
            pqc = s1tile([128, QB], f"pqc{h}_{qb}")
            for l in range(NLC):
                _lbl(nc.tensor.matmul(pqc[:], wuq_t[l][:, h * DH:(h + 1) * DH],
                                 cq[l][:], start=(l == 0), stop=(l == NLC - 1)), "s2_qc")
            nc.vector.tensor_copy(qct[h][:], pqc[:])
        # k_C per head into persistent K cache
        kc_rest(qb, ckv)
        return qct

    def kc_rest(qb, ckv):
        qsl = slice(qb * QB, (qb + 1) * QB)
        for h in range(HC):
            pkc = s1tile([128, QB], f"pkc{h}_{qb}")
            for l in range(NLC):
                _lbl(nc.tensor.matmul(pkc[:], wuk_t[l][:, h * DH:(h + 1) * DH],
                                 ckv[l][:], start=(l == 0), stop=(l == NLC - 1)), "s2_kc")
            (nc.vector.tensor_copy if h == 0 else nc.scalar.copy)(
                kct[h][:, qsl], pkc[:])
        # V chunks (natural layout, both heads packed)
        for sc in range(KPB):
            k = qb * KPB + sc
            pv = s1tile([128, HC * DH], f"pvv{k}")
            for l in range(NLC):
                _lbl(nc.tensor.matmul(pv[:], ckv[l][:, sc * 128:(sc + 1) * 128],
                                 wuv_t[l][:], start=(l == 0), stop=(l == NLC - 1)), "s2_v")
            (nc.vector.tensor_copy if sc % 2 == 0 else nc.scalar.copy)(
                vt[k][:], pv[:])

    def attn_both(qb, qct, qrt):
        """Both heads interleaved per key chunk: 2x PE density per chain step."""
        nkc = KPB * (qb + 1)
        pat = [ps_at.tile([128, QB], F32, tag="at", name=f"pat{h}_{qb}")
               for h in range(HC)]
        psums = [ps_sm.tile([128, QB], F32, tag="smrb", name=f"psums{h}_{qb}")
                 for h in range(HC)]
        pend = []  # (h, kc, off, pt) awaiting PV+sums

        def flush(last):
            h, kc, off, pt = pend.pop(0)
            _lbl(nc.tensor.matmul(psums[h][:, off:], o128_t[:], pt[:, off:],
                                  start=(kc == 0), stop=last,
                                  skip_group_check=True), "sum")
            _lbl(nc.tensor.matmul(pat[h][:, off:],
                                  vt[kc][:, h * DH:(h + 1) * DH],
                                  pt[:, off:], start=(kc == 0), stop=last,
                                  skip_group_check=True), "pv")

        for kc in range(nkc):
            off = 128 * (kc - KPB * qb) if kc >= KPB * qb else 0
            w = QB - off
            ksl = slice(kc * 128, (kc + 1) * 128)
            for h in range(HC):
                ps_s = s1tile([128, QB], f"s{h}_{qb}_{kc}")
                _lbl(nc.tensor.matmul(ps_s[:, off:], kct[h][:, ksl],
                                      qct[h][:, off:], start=True, stop=False,
                                      skip_group_check=True), "qk_c")
                _lbl(nc.tensor.matmul(ps_s[:, off:], krt[:, ksl],
                                      qrt[h][:, off:], start=False, stop=True,
                                      skip_group_check=True), "qk_r")
                if len(pend) >= 2:
                    flush(False)
                et = smp.tile([128, QB], F32, tag="et", bufs=5,
                              name=f"et{h}_{qb}_{kc}")
                nc.scalar.activation(et[:, off:], ps_s[:, off:], AF.Exp,
                                     scale=SCALE)
                pt = smp.tile([128, QB], BF16, tag="pt", bufs=5,
                              name=f"pt{h}_{qb}_{kc}")
                if kc >= KPB * qb:  # diagonal: clip+mask window, clip rest
                    ctw = smp.tile([128, 128], F32, tag="ctw", bufs=3,
                                   name=f"ctw{h}_{qb}_{kc}")
                    nc.vector.tensor_scalar(ctw[:], et[:, off:off + 128],
                                            E_HI, E_LO, op0=OP.min, op1=OP.max)
                    nc.vector.tensor_tensor(pt[:, off:off + 128], ctw[:],
                                            mask_t[:], op=OP.mult)
                    if w > 128:
                        nc.vector.tensor_scalar(pt[:, off + 128:],
                                                et[:, off + 128:], E_HI, E_LO,
                                                op0=OP.min, op1=OP.max)
                else:
                    nc.vector.tensor_scalar(pt[:], et[:], E_HI, E_LO,
                                            op0=OP.min, op1=OP.max)
                pend.append((h, kc, off, pt))
        while len(pend) > 2:
            flush(False)
        while pend:
            flush(True)
        return pat, psums

    def attn_sum(qb, h, psums):  # psums: [128,QB], denom bcast on partitions
        rcr = smp.tile([128, QB], F32, tag="rcr", bufs=2, name=f"rcr{h}_{qb}")
        nc.vector.reciprocal_approx_fast(rcr[:], psums[:])
        return rcr

    def attn_norm(qb, h, pat, rcr, attn_n):
        nc.vector.tensor_tensor(attn_n[:], pat[:], rcr[:], op=OP.mult)

    def stage5(qb, attn_n):
        qsl = slice(qb * QB, (qb + 1) * QB)
        for m in range(NMC):
            po = s1tile([128, QB], f"po{m}_{qb}")
            for h in range(HC):
                _lbl(nc.tensor.matmul(po[:], wo_t[h][:, m * 128:(m + 1) * 128],
                                 attn_n[h][:], start=(h == 0),
                                 stop=(h == HC - 1)), "s5")
            ob = o5p.tile([128, QB], BF16, tag="ob", name=f"ob{m}_{qb}")
            (nc.vector.tensor_copy if m % 2 == 0 else nc.scalar.copy)(
                ob[:], po[:])
            nc.sync.dma_start(d["outT"][m * 128:(m + 1) * 128, qsl], ob[:])

    # ---- software-pipelined main loop ----
    def first_hooks():
        emit_wdkv_dmas()

    cq, ckv, cs1s, cs2s = stage1(0, mid_hook=first_hooks)
    emit_proj_dmas()
    emit_wo_dmas()
    qct = stage2(0, cq, ckv, cs1s, cs2s)
    for qb in range(NQB):
        attn_n = [prj.tile([128, QB], BF16, tag=f"an{h}", name=f"an{h}_{qb}")
                  for h in range(HC)]
        pat, psums = attn_both(qb, qct, qrt)
        rcr0 = attn_sum(qb, 0, psums[0])
        rcr1 = attn_sum(qb, 1, psums[1])
        if qb < NQB - 1:
            cq, ckv, cs1s, cs2s = stage1(qb + 1)  # hides the reciprocal chains
        attn_norm(qb, 0, pat[0], rcr0, attn_n[0][:])
        attn_norm(qb, 1, pat[1], rcr1, attn_n[1][:])
        if qb < NQB - 1:
            # next block's projections ahead of stage5: their rope /
            # copy chains drain under stage-5 PE work, so attention
            # (qb+1) starts with q ready
            qct = stage2(qb + 1, cq, ckv, cs1s, cs2s)
        stage5(qb, attn_n)


def _prep_inputs(x, W_DQ, W_UQ, W_QR, W_DKV, W_UK, W_UV, W_KR, W_O):
    """Host-side sharding + layout prep. Returns list of 8 in_maps."""
    import ml_dtypes
    f32 = np.float32
    bf16 = ml_dtypes.bfloat16
    xT = np.ascontiguousarray(x[0].T).astype(bf16)
    perm = np.concatenate([np.arange(0, DHR, 2), np.arange(1, DHR, 2)])
    wdqT = np.ascontiguousarray(W_DQ.T).astype(bf16)
    wdkvT = np.ascontiguousarray(W_DKV.T).astype(bf16)
    wkrT = np.ascontiguousarray(np.concatenate([W_KR.T[:, perm]] * 2, axis=1)).astype(bf16)

    # rope tables (transposed, permuted-channel layout)
    pos = np.arange(S, dtype=np.float64)
    inv = THETA ** (-np.arange(0, DHR, 2, dtype=np.float64) / DHR)  # (32,)
    ang = inv[:, None] * pos[None, :]                               # (32, S)
    cosv = np.cos(ang).astype(f32)
    sinv = np.sin(ang).astype(f32)
    cs1 = np.ascontiguousarray(np.concatenate([cosv, cosv], axis=0))
    cs2 = np.ascontiguousarray(np.concatenate([-sinv, sinv], axis=0))

    # triangle mask for the 128-wide diagonal window: allow k <= q
    kk = np.arange(128)[:, None]
    qq = np.arange(128)[None, :]
    masktri = np.ascontiguousarray((kk <= qq).astype(f32))

    shared = {
        "xT": xT, "wdqT": wdqT, "wdkvT": wdkvT, "wkrT": wkrT,
        "masktri": masktri, "cs1": cs1, "cs2": cs2,
        "ones128": np.ones((128, 128), bf16), "ones1": np.ones((1, 128), f32),
        "zeros64": np.zeros((DHR, S), bf16),
    }
    in_maps = []
    for c in range(N_CORES):
        hs = [c * HC + h for h in range(HC)]
        wuqT = np.concatenate(
            [W_UQ[h * DH:(h + 1) * DH, :].T for h in hs], axis=1)
        wqrT = np.concatenate(
            [W_QR[h * DHR:(h + 1) * DHR, :].T[:, perm] for h in hs], axis=1)
        wukT = np.concatenate(
            [W_UK[h * DH:(h + 1) * DH, :].T for h in hs], axis=1)
        wuvT = np.concatenate(
            [W_UV[h * DH:(h + 1) * DH, :].T for h in hs], axis=1)
        woT = np.concatenate(
            [W_O[:, h * DH:(h + 1) * DH].T for h in hs], axis=0)
        in_maps.append({
            **shared,
            "wuqT": np.ascontiguousarray(wuqT).astype(bf16),
            "wqrT": np.ascontiguousarray(wqrT).astype(bf16),
            "wukT": np.ascontiguousarray(wukT).astype(bf16),
            "wuvT": np.ascontiguousarray(wuvT).astype(bf16),
            "woT": np.ascontiguousarray(woT).astype(bf16),
        })
    return in_maps


def kernel(**inputs):
    global LAST_EXEC_TIME_NS, LAST_RESULTS
    if "nc" not in _CACHE:
        _CACHE["nc"] = _build()
    nc = _CACHE["nc"]
    in_maps = _prep_inputs(**{k: np.asarray(v) for k, v in inputs.items()})
    kwargs = dict(TRACE_KWARGS)
    if TRACE:
        kwargs["trace"] = True
    res = run_bass_kernel_spmd(nc, in_maps, core_ids=list(range(N_CORES)),
                               **kwargs)
    LAST_EXEC_TIME_NS = res.exec_time_ns
    LAST_RESULTS = res
    acc = np.zeros((DM, S), np.float64)
    for c in range(N_CORES):
        acc += res.results[c]["outT"].astype(np.float64)
    return np.ascontiguousarray(acc.T[None]).astype(np.float32)



# revision 45
# speedup vs baseline: 1.0124x; 1.0124x over previous
"""Multi-Head Latent Attention (MLA) Trainium2 kernel, 8-core head-sharded.

Layout: all matmuls run with the contraction dim on partitions
("transposed world"); x and every weight are pre-transposed on the host.
Heads are sharded 2-per-core; each core emits a bf16 partial out.T (its
heads' contribution to the output projection), summed and transposed on
the host (rel err ~4e-3, harness gate 2e-2).

Precision: bf16 operands everywhere on the PE (same 1 cycle/column rate
as fp32r but fast FWL weight loads and half the DMA/SBUF); all PSUM
accumulation is fp32. W_DQ is SBUF-resident in bf16 (no weight stream).
k_R/q_R are zero-padded to 128 partitions (64-row fp32r moving operands
ran at half rate; also keeps bf16 FWL eligible), and W_KR columns are
host-duplicated so the kr matmul has a full 128-wide stationary. The
softmax denominator is a matmul against an all-ones [128,128] stationary,
which lands the row-sum broadcast across all 128 PSUM partitions - the
reciprocal (reciprocal_approx_fast) is then multiplied in directly with
no broadcast matmul.

Pipeline per query block qb: stage2(qb, q-rope first) -> attention(qb)
-> stage1(qb+1) -> stage5(qb), so the reciprocal chain and next-block
DMAs hide under stage-1 matmuls. DMA issue is spread across the sync /
gpsimd / scalar queues to avoid head-of-line blocking of the out-DMAs.
"""
import sys

sys.path.insert(0, "/opt/trn_rl_repo")

import numpy as np

import concourse.bass as bass
import concourse.tile as tile
from concourse import bacc, mybir
from concourse.bass_utils import run_bass_kernel_spmd

F32 = mybir.dt.float32
F32R = mybir.dt.float32r
BF16 = mybir.dt.bfloat16
AF = mybir.ActivationFunctionType
OP = mybir.AluOpType

N_CORES = 8
S = 2048          # sequence length
DM = 2048         # d_model
DL = 512          # d_latent
H = 16            # total heads
HC = H // N_CORES  # heads per core (2)
DH = 128          # head dim (content)
DHR = 64          # head dim (rope)
QB = 512          # query block
NQB = S // QB     # 4
KPB = QB // 128   # key chunks per query block (4)
NMC = DM // 128   # 16 model chunks
NLC = DL // 128   # 4 latent chunks
NKC = S // 128    # 16 key chunks
THETA = 10000.0

SCALE = float(1.0 / np.sqrt(np.float32(DH + DHR)))
E_HI = float(np.exp(np.float64(80.0) * SCALE))
E_LO = float(np.exp(np.float64(-80.0) * SCALE))

# Set by test.py to profile; harness path leaves these untouched.
TRACE = False
TRACE_KWARGS = {}
LAST_EXEC_TIME_NS = None
LAST_RESULTS = None

_CACHE = {}
MM_LABELS = {}


def _lbl(inst, label):
    try:
        MM_LABELS[inst.ins.name] = label
    except Exception:
        try:
            MM_LABELS[inst.name] = label
        except Exception:
            pass
    return inst


def _build():
    nc = bacc.Bacc("TRN2", target_bir_lowering=False, debug=False,
                   enable_asserts=True, num_devices=N_CORES)

    def din(name, shape, dt=F32R):
        return nc.dram_tensor(name, shape, dt, kind="ExternalInput").ap()

    d = {
        "xT": din("xT", [DM, S], BF16),
        "wdqT": din("wdqT", [DM, DL], BF16),
        "wdkvT": din("wdkvT", [DM, DL], BF16),
        "wkrT": din("wkrT", [DM, 128], BF16),
        "wuqT": din("wuqT", [DL, HC * DH], BF16),
        "wqrT": din("wqrT", [DL, HC * DHR], BF16),
        "wukT": din("wukT", [DL, HC * DH], BF16),
        "wuvT": din("wuvT", [DL, HC * DH], BF16),
        "woT": din("woT", [HC * DH, DM], BF16),
        "ones128": din("ones128", [128, 128], BF16),
        "ones1": din("ones1", [1, 128]),
        "masktri": din("masktri", [128, 128], F32),
        "zeros64": din("zeros64", [64, S], BF16),
        "cs1": din("cs1", [DHR, S], F32),
        "cs2": din("cs2", [DHR, S], F32),
        "outT": nc.dram_tensor("outT", [DM, S], BF16,
                               kind="ExternalOutput").ap(),
    }
    with tile.TileContext(nc) as tc:
        import contextlib
        with contextlib.ExitStack() as ctx:
            _kernel_body(ctx, tc, nc, d)
    nc.compile()
    return nc


def _kernel_body(ctx, tc, nc, d):
    wts = ctx.enter_context(tc.tile_pool(name="wts", bufs=1))
    kvp = ctx.enter_context(tc.tile_pool(name="kvp", bufs=1))
    xtp = ctx.enter_context(tc.tile_pool(name="xtp", bufs=1))
    lat = ctx.enter_context(tc.tile_pool(name="lat", bufs=1))
    prj = ctx.enter_context(tc.tile_pool(name="prj", bufs=1))
    smp = ctx.enter_context(tc.tile_pool(name="smp", bufs=1))
    o5p = ctx.enter_context(tc.tile_pool(name="o5p", bufs=8))
    # PSUM: stage-1 dedicated (3) + work rotation (2) + attn (2) + sums (1)
    ps_s1 = ctx.enter_context(tc.tile_pool(name="ps_s1", bufs=1, space="PSUM"))
    ps_at = ctx.enter_context(tc.tile_pool(name="ps_at", bufs=2, space="PSUM"))
    ps_sm = ctx.enter_context(tc.tile_pool(name="ps_sm", bufs=2, space="PSUM"))

    s1rot = [0]

    def s1tile(shape, name):
        t = ps_s1.tile(shape, F32, tag=f"s1{s1rot[0] % 4}", name=name)
        s1rot[0] += 1
        return t

    # ---- stage-1 weights first: pass 0 consumes wkr[m]+wdq[m] at
    # ~640ns/m, so issue them interleaved per m across two queues ----
    wkr_t = [wts.tile([128, 128], BF16, name=f"wkr{m}") for m in range(NMC)]
    wdq_t = [wts.tile([128, DL], BF16, name=f"wdq{m}") for m in range(NMC)]
    for m in range(NMC):
        e = nc.gpsimd if m % 2 == 0 else nc.scalar
        e.dma_start(wkr_t[m][:], d["wkrT"][m * 128:(m + 1) * 128, :])
        e.dma_start(wdq_t[m][:], d["wdqT"][m * 128:(m + 1) * 128, :])

    # small persistent loads (not needed until attention)
    o128_t = wts.tile([128, 128], BF16, name="o128")
    o1_t = wts.tile([1, 128], F32R, name="o1")
    nc.scalar.dma_start(o128_t[:], d["ones128"][:, :])
    nc.scalar.dma_start(o1_t[:], d["ones1"][:, :])
    mask_t = wts.tile([128, 128], F32, name="masktri")
    nc.scalar.dma_start(mask_t[:], d["masktri"][:, :])
    wdkv_t = [wts.tile([128, DL], BF16, name=f"wdkv{m}") for m in range(NMC)]
    wuq_t = [wts.tile([128, HC * DH], BF16, name=f"wuq{l}") for l in range(NLC)]
    wqr_t = [wts.tile([128, HC * DHR], BF16, name=f"wqr{l}") for l in range(NLC)]
    wuk_t = [wts.tile([128, HC * DH], BF16, name=f"wuk{l}") for l in range(NLC)]
    wuv_t = [wts.tile([128, HC * DH], BF16, name=f"wuv{l}") for l in range(NLC)]
    wo_t = [wts.tile([128, DM], BF16, name=f"wo{h}") for h in range(HC)]

    def emit_wdkv_dmas():
        for m in range(NMC):
            nc.gpsimd.dma_start(wdkv_t[m][:], d["wdkvT"][m * 128:(m + 1) * 128, :])

    def emit_proj_dmas():
        for l in range(NLC):
            nc.gpsimd.dma_start(wuk_t[l][:], d["wukT"][l * 128:(l + 1) * 128, :])
            nc.gpsimd.dma_start(wuv_t[l][:], d["wuvT"][l * 128:(l + 1) * 128, :])
            nc.gpsimd.dma_start(wuq_t[l][:], d["wuqT"][l * 128:(l + 1) * 128, :])
            nc.gpsimd.dma_start(wqr_t[l][:], d["wqrT"][l * 128:(l + 1) * 128, :])

    def emit_wo_dmas():
        for h in range(HC):
            nc.gpsimd.dma_start(wo_t[h][:], d["woT"][h * 128:(h + 1) * 128, :])

    # ---- persistent per-sequence state ----
    kct = [kvp.tile([128, S], BF16, name=f"kct{h}") for h in range(HC)]
    # krt/qrt are zero-padded to 128 partitions: a 64-partition moving
    # operand runs fp32r matmuls at half rate.
    krt = kvp.tile([128, S], BF16, name="krt")
    nc.scalar.dma_start(krt[DHR:128, :], d["zeros64"][:, :])
    qrt = [kvp.tile([128, QB], BF16, name=f"qrt{h}") for h in range(HC)]
    for h in range(HC):
        nc.scalar.dma_start(qrt[h][DHR:128, :], d["zeros64"][:, 0:QB])
    vt = [kvp.tile([128, HC * DH], BF16, name=f"vt{k}") for k in range(NKC)]



    def rope(raw_pt, out_ap, cs1s, cs2s, tag):
        """raw_pt: PSUM tile holding [64, QB] pre-rope rows; out_ap:
        bf16 dest [64, QB]. Swap-halves DMA reads PSUM directly; the cs2
        leg runs on gpsimd so the two products overlap."""
        raw = smp.tile([DHR, QB], F32, tag="rope_srcc", name=f"rc_{tag}")
        nc.scalar.copy(raw[:], raw_pt[0:DHR, :])
        rsw = smp.tile([DHR, QB], F32, tag="rope_swp", name=f"rs_{tag}")
        nc.sync.dma_start(rsw[0:32, :], raw[32:64, :])
        nc.sync.dma_start(rsw[32:64, :], raw[0:32, :])
        rawm = smp.tile([DHR, QB], F32, tag="rope_raw", name=f"rr_{tag}")
        nc.vector.tensor_tensor(rawm[:], raw_pt[0:DHR, :], cs1s[:], op=OP.mult)
        nc.gpsimd.tensor_tensor(rsw[:], rsw[:], cs2s[:], op=OP.mult)
        nc.vector.tensor_tensor(out_ap, rawm[:], rsw[:], op=OP.add)

    def stage1(qb, mid_hook=None):
        """Latents in 4 mc-major passes; x and all stage-1 weights are
        bf16 (resident W_DQ, no weight streaming)."""
        qsl = slice(qb * QB, (qb + 1) * QB)
        xt = [xtp.tile([128, QB], BF16, tag=f"xt{m}", name=f"xt{m}_{qb}")
              for m in range(NMC)]
        for m in range(NMC):
            nc.sync.dma_start(xt[m][:], d["xT"][m * 128:(m + 1) * 128, qsl])
        cs1s = smp.tile([DHR, QB], F32, tag="cs1s", bufs=1, name=f"cs1s{qb}")
        cs2s = smp.tile([DHR, QB], F32, tag="cs2s", bufs=1, name=f"cs2s{qb}")
        nc.gpsimd.dma_start(cs1s[:], d["cs1"][:, qsl])
        nc.gpsimd.dma_start(cs2s[:], d["cs2"][:, qsl])

        ckv = [lat.tile([128, QB], BF16, tag=f"ckv{l}", name=f"ckv{l}_{qb}")
               for l in range(NLC)]
        cq = [lat.tile([128, QB], BF16, tag=f"cq{l}", name=f"cq{l}_{qb}")
              for l in range(NLC)]
        eng_tgl = [0]

        def copy_out(dst, src):
            (nc.vector.tensor_copy if eng_tgl[0] % 2 == 0
             else nc.scalar.copy)(dst, src)
            eng_tgl[0] += 1

        plan = [
            [("kr", None), ("cq", 0), ("cq", 1)],
            [("cq", 2), ("cq", 3)],
            [("ckv", 0), ("ckv", 1)],
            [("ckv", 2), ("ckv", 3)],
        ]
        for pi, groups in enumerate(plan):
            pts = []
            for gi, (kind, idx) in enumerate(groups):
                pts.append(s1tile([128, QB], f"p{pi}{gi}_{qb}"))
            for m in range(NMC):
                for gi, (kind, idx) in enumerate(groups):
                    if kind == "kr":
                        st_ap, label = wkr_t[m][:], "s1_kr"
                    elif kind == "cq":
                        st_ap = wdq_t[m][:, idx * 128:(idx + 1) * 128]
                        label = "s1_cq"
                    else:
                        st_ap = wdkv_t[m][:, idx * 128:(idx + 1) * 128]
                        label = "s1_ckv"
                    _lbl(nc.tensor.matmul(pts[gi][:], st_ap, xt[m][:],
                                          start=(m == 0),
                                          stop=(m == NMC - 1)), label)
            for gi, (kind, idx) in enumerate(groups):
                if kind == "kr":
                    rope(pts[gi], krt[0:DHR, qsl], cs1s, cs2s, f"kr{qb}")
                elif kind == "cq":
                    copy_out(cq[idx][:], pts[gi][:])
                else:
                    copy_out(ckv[idx][:], pts[gi][:])
            if pi == 1 and mid_hook is not None:
                mid_hook()
        return cq, ckv, cs1s, cs2s

    def stage2(qb, cq, ckv, cs1s, cs2s):
        qsl = slice(qb * QB, (qb + 1) * QB)
        # q_C / q_R per head
        qct = [prj.tile([128, QB], BF16, tag=f"qct{h}", bufs=2, name=f"qct{h}_{qb}")
               for h in range(HC)]
        for h in range(HC):
            pqr = s1tile([DHR, QB], f"pqr{h}_{qb}")
            for l in range(NLC):
                _lbl(nc.tensor.matmul(pqr[:], wqr_t[l][:, h * DHR:(h + 1) * DHR],
                                 cq[l][:], start=(l == 0), stop=(l == NLC - 1)), "s2_qr")
            rope(pqr, qrt[h][0:DHR, :], cs1s, cs2s, f"qr{h}_{qb}")
        for h in range(HC):
            pqc = s1tile([128, QB], f"pqc{h}_{qb}")
            for l in range(NLC):
                _lbl(nc.tensor.matmul(pqc[:], wuq_t[l][:, h * DH:(h + 1) * DH],
                                 cq[l][:], start=(l == 0), stop=(l == NLC - 1)), "s2_qc")
            nc.vector.tensor_copy(qct[h][:], pqc[:])
        # k_C per head into persistent K cache
        kc_rest(qb, ckv)
        return qct

    def kc_rest(qb, ckv):
        qsl = slice(qb * QB, (qb + 1) * QB)
        for h in range(HC):
            pkc = s1tile([128, QB], f"pkc{h}_{qb}")
            for l in range(NLC):
                _lbl(nc.tensor.matmul(pkc[:], wuk_t[l][:, h * DH:(h + 1) * DH],
                                 ckv[l][:], start=(l == 0), stop=(l == NLC - 1)), "s2_kc")
            (nc.vector.tensor_copy if h == 0 else nc.scalar.copy)(
                kct[h][:, qsl], pkc[:])
        # V chunks (natural layout, both heads packed)
        for sc in range(KPB):
            k = qb * KPB + sc
            pv = s1tile([128, HC * DH], f"pvv{k}")
            for l in range(NLC):
                _lbl(nc.tensor.matmul(pv[:], ckv[l][:, sc * 128:(sc + 1) * 128],
                                 wuv_t[l][:], start=(l == 0), stop=(l == NLC - 1)), "s2_v")
            (nc.vector.tensor_copy if sc % 2 == 0 else nc.scalar.copy)(
                vt[k][:], pv[:])

    def attn_both(qb, qct, qrt):
        """Both heads interleaved per key chunk: 2x PE density per chain step."""
        nkc = KPB * (qb + 1)
        pat = [ps_at.tile([128, QB], F32, tag="at", name=f"pat{h}_{qb}")
               for h in range(HC)]
        psums = [ps_sm.tile([128, QB], F32, tag="smrb", name=f"psums{h}_{qb}")
                 for h in range(HC)]
        pend = []  # (h, kc, off, pt) awaiting PV+sums

        def flush(last):
            h, kc, off, pt = pend.pop(0)
            _lbl(nc.tensor.matmul(psums[h][:, off:], o128_t[:], pt[:, off:],
                                  start=(kc == 0), stop=last,
                                  skip_group_check=True), "sum")
            _lbl(nc.tensor.matmul(pat[h][:, off:],
                                  vt[kc][:, h * DH:(h + 1) * DH],
                                  pt[:, off:], start=(kc == 0), stop=last,
                                  skip_group_check=True), "pv")

        for kc in range(nkc):
            off = 128 * (kc - KPB * qb) if kc >= KPB * qb else 0
            w = QB - off
            ksl = slice(kc * 128, (kc + 1) * 128)
            for h in range(HC):
                ps_s = s1tile([128, QB], f"s{h}_{qb}_{kc}")
                _lbl(nc.tensor.matmul(ps_s[:, off:], kct[h][:, ksl],
                                      qct[h][:, off:], start=True, stop=False,
                                      skip_group_check=True), "qk_c")
                _lbl(nc.tensor.matmul(ps_s[:, off:], krt[:, ksl],
                                      qrt[h][:, off:], start=False, stop=True,
                                      skip_group_check=True), "qk_r")
                if len(pend) >= 2:
                    flush(False)
                et = smp.tile([128, QB], F32, tag="et", bufs=5,
                              name=f"et{h}_{qb}_{kc}")
                nc.scalar.activation(et[:, off:], ps_s[:, off:], AF.Exp,
                                     scale=SCALE)
                pt = smp.tile([128, QB], BF16, tag="pt", bufs=5,
                              name=f"pt{h}_{qb}_{kc}")
                if kc >= KPB * qb:  # diagonal: clip+mask window, clip rest
                    ctw = smp.tile([128, 128], F32, tag="ctw", bufs=3,
                                   name=f"ctw{h}_{qb}_{kc}")
                    nc.vector.tensor_scalar(ctw[:], et[:, off:off + 128],
                                            E_HI, E_LO, op0=OP.min, op1=OP.max)
                    nc.vector.tensor_tensor(pt[:, off:off + 128], ctw[:],
                                            mask_t[:], op=OP.mult)
                    if w > 128:
                        nc.vector.tensor_scalar(pt[:, off + 128:],
                                                et[:, off + 128:], E_HI, E_LO,
                                                op0=OP.min, op1=OP.max)
                else:
                    nc.vector.tensor_scalar(pt[:], et[:], E_HI, E_LO,
                                            op0=OP.min, op1=OP.max)
                pend.append((h, kc, off, pt))
        while len(pend) > 2:
            flush(False)
        while pend:
            flush(True)
        return pat, psums

    def attn_sum(qb, h, psums):  # psums: [128,QB], denom bcast on partitions
        rcr = smp.tile([128, QB], F32, tag="rcr", bufs=2, name=f"rcr{h}_{qb}")
        nc.vector.reciprocal_approx_fast(rcr[:], psums[:])
        return rcr

    def attn_norm(qb, h, pat, rcr, attn_n):
        nc.vector.tensor_tensor(attn_n[:], pat[:], rcr[:], op=OP.mult)

    def stage5(qb, attn_n):
        qsl = slice(qb * QB, (qb + 1) * QB)
        for m in range(NMC):
            po = s1tile([128, QB], f"po{m}_{qb}")
            for h in range(HC):
                _lbl(nc.tensor.matmul(po[:], wo_t[h][:, m * 128:(m + 1) * 128],
                                 attn_n[h][:], start=(h == 0),
                                 stop=(h == HC - 1)), "s5")
            ob = o5p.tile([128, QB], BF16, tag="ob", name=f"ob{m}_{qb}")
            (nc.vector.tensor_copy if m % 2 == 0 else nc.scalar.copy)(
                ob[:], po[:])
            nc.sync.dma_start(d["outT"][m * 128:(m + 1) * 128, qsl], ob[:])

    # ---- software-pipelined main loop ----
    def first_hooks():
        emit_wdkv_dmas()

    cq, ckv, cs1s, cs2s = stage1(0, mid_hook=first_hooks)
    emit_proj_dmas()
    emit_wo_dmas()
    qct = stage2(0, cq, ckv, cs1s, cs2s)
    for qb in range(NQB):
        attn_n = [prj.tile([128, QB], BF16, tag=f"an{h}", name=f"an{h}_{qb}")
                  for h in range(HC)]
        pat, psums = attn_both(qb, qct, qrt)
        rcr0 = attn_sum(qb, 0, psums[0])
        rcr1 = attn_sum(qb, 1, psums[1])
        if qb < NQB - 1:
            cq, ckv, cs1s, cs2s = stage1(qb + 1)  # hides the reciprocal chains
        attn_norm(qb, 0, pat[0], rcr0, attn_n[0][:])
        attn_norm(qb, 1, pat[1], rcr1, attn_n[1][:])
        if qb < NQB - 1:
            # next block's projections ahead of stage5: their rope /
            # copy chains drain under stage-5 PE work, so attention
            # (qb+1) starts with q ready
            qct = stage2(qb + 1, cq, ckv, cs1s, cs2s)
        stage5(qb, attn_n)


def _prep_inputs(x, W_DQ, W_UQ, W_QR, W_DKV, W_UK, W_UV, W_KR, W_O):
    """Host-side sharding + layout prep. Returns list of 8 in_maps."""
    import ml_dtypes
    f32 = np.float32
    bf16 = ml_dtypes.bfloat16
    xT = np.ascontiguousarray(x[0].T).astype(bf16)
    perm = np.concatenate([np.arange(0, DHR, 2), np.arange(1, DHR, 2)])
    wdqT = np.ascontiguousarray(W_DQ.T).astype(bf16)
    wdkvT = np.ascontiguousarray(W_DKV.T).astype(bf16)
    wkrT = np.ascontiguousarray(np.concatenate([W_KR.T[:, perm]] * 2, axis=1)).astype(bf16)

    # rope tables (transposed, permuted-channel layout)
    pos = np.arange(S, dtype=np.float64)
    inv = THETA ** (-np.arange(0, DHR, 2, dtype=np.float64) / DHR)  # (32,)
    ang = inv[:, None] * pos[None, :]                               # (32, S)
    cosv = np.cos(ang).astype(f32)
    sinv = np.sin(ang).astype(f32)
    cs1 = np.ascontiguousarray(np.concatenate([cosv, cosv], axis=0))
    cs2 = np.ascontiguousarray(np.concatenate([-sinv, sinv], axis=0))

    # triangle mask for the 128-wide diagonal window: allow k <= q
    kk = np.arange(128)[:, None]
    qq = np.arange(128)[None, :]
    masktri = np.ascontiguousarray((kk <= qq).astype(f32))

    shared = {
        "xT": xT, "wdqT": wdqT, "wdkvT": wdkvT, "wkrT": wkrT,
        "masktri": masktri, "cs1": cs1, "cs2": cs2,
        "ones128": np.ones((128, 128), bf16), "ones1": np.ones((1, 128), f32),
        "zeros64": np.zeros((DHR, S), bf16),
    }
    in_maps = []
    for c in range(N_CORES):
        hs = [c * HC + h for h in range(HC)]
        wuqT = np.concatenate(
            [W_UQ[h * DH:(h + 1) * DH, :].T for h in hs], axis=1)
        wqrT = np.concatenate(
            [W_QR[h * DHR:(h + 1) * DHR, :].T[:, perm] for h in hs], axis=1)
        wukT = np.concatenate(
            [W_UK[h * DH:(h + 1) * DH, :].T for h in hs], axis=1)
        wuvT = np.concatenate(
            [W_UV[h * DH:(h + 1) * DH, :].T for h in hs], axis=1)
        woT = np.concatenate(
            [W_O[:, h * DH:(h + 1) * DH].T for h in hs], axis=0)
        in_maps.append({
            **shared,
            "wuqT": np.ascontiguousarray(wuqT).astype(bf16),
            "wqrT": np.ascontiguousarray(wqrT).astype(bf16),
            "wukT": np.ascontiguousarray(wukT).astype(bf16),
            "wuvT": np.ascontiguousarray(wuvT).astype(bf16),
            "woT": np.ascontiguousarray(woT).astype(bf16),
        })
    return in_maps


def kernel(**inputs):
    global LAST_EXEC_TIME_NS, LAST_RESULTS
    if "nc" not in _CACHE:
        _CACHE["nc"] = _build()
    nc = _CACHE["nc"]
    in_maps = _prep_inputs(**{k: np.asarray(v) for k, v in inputs.items()})
    kwargs = dict(TRACE_KWARGS)
    if TRACE:
        kwargs["trace"] = True
    res = run_bass_kernel_spmd(nc, in_maps, core_ids=list(range(N_CORES)),
                               **kwargs)
    LAST_EXEC_TIME_NS = res.exec_time_ns
    LAST_RESULTS = res
    acc = np.zeros((DM, S), np.float64)
    for c in range(N_CORES):
        acc += res.results[c]["outT"].astype(np.float64)
    return np.ascontiguousarray(acc.T[None]).astype(np.float32)

